# revision 13
# baseline (speedup 1.0000x reference)
"""Trainium2 Bass kernel for a causal single-head attention module (v4).

reference computation (per batch b):
    q = x @ Wq; k = x @ Wk; v = x @ Wv          # [s, 128]
    att = softmax(mask(q @ k.T / sqrt(1024)))   # causal
    out = att @ v                               # [s, 128]

Shapes: x [4, 4096, 1024] f32, W* [1024, 128] f32.

Distribution: 8 NeuronCores, 2 per batch.  The 8 sequence blocks (512 rows
each) of a batch are split between its two cores: core 2b owns blocks
{1,3,5,7}, core 2b+1 owns {0,2,4,6}.  This interleaving balances the causal
triangle AND makes the per-core instruction graph identical (SPMD): every
core runs 4 q-tiles whose key-group counts are {2,4,6,8}; the odd core's
extra (non-causal) key group per tile is zeroed via a per-core input scalar.

v4 restructure vs v3: the peer-half K/V projections are software-pipelined
INTO the attention tiles.  Attention runs in tile order 0..3; peer key
group 4+i is projected just before tile i needs it, and the following
group's projection matmuls are emitted as PE filler between each slot's
St and AV so the PE streams through the exp waits instead of stalling.
This also spreads the Scalar engine's exp stream (~46us) across the whole
PE timeline instead of concentrating it in a trailing attention phase.
First-DMA granularity is finer (per-chunk W slices, 512-col first xt
pieces) so the first projection matmul issues ~4us earlier.
PSUM: St halves 2x2 banks + ot 2 + sums 1 + proj 1 = 8 banks.
"""

import os
import ml_dtypes
import numpy as np

import concourse.bass as bass
import concourse.bacc as bacc
import concourse.mybir as mybir
import concourse.tile as tile
from concourse.bass_utils import run_bass_kernel_spmd
from concourse.tile_rust import add_dep_helper

F32 = mybir.dt.float32
BF16 = mybir.dt.bfloat16

BATCH = 4
SEQ = 4096
EMB = 1024
DK = 128
P = 128
NCORES = 8
SCALE = 1.0 / float(np.sqrt(EMB))

NBLK = 8
HEAVY_BLOCKS = [1, 3, 5, 7]  # core 2b   (exact causal fit)
LIGHT_BLOCKS = [0, 2, 4, 6]  # core 2b+1 (one padded key-group per tile)


def build_nc(seq: int = SEQ):
    blk = seq // NBLK          # 512
    sub = blk // P             # 4 key subtiles per group
    kcols = 4 * blk            # own rows per core (2048)
    xcols = 8 * blk            # own + peer rows (K/V replicated)
    emb_c = EMB // P           # 8 contraction chunks
    nch = kcols // blk         # 4 projection column chunks of 512

    nc = bacc.Bacc("TRN2", target_bir_lowering=False, debug=False,
                   num_devices=NCORES)

    xt = nc.dram_tensor("xt", [EMB, xcols], BF16, kind="ExternalInput")
    wq = nc.dram_tensor("wq", [P, emb_c, DK], BF16, kind="ExternalInput")
    wk = nc.dram_tensor("wk", [P, emb_c, DK], BF16, kind="ExternalInput")
    wv = nc.dram_tensor("wv", [P, emb_c, DK], BF16, kind="ExternalInput")
    pad = nc.dram_tensor("pad", [P, 1], F32, kind="ExternalInput")
    out_o = nc.dram_tensor("out_o", [P, 4 * blk], F32, kind="ExternalOutput")
    out_s = nc.dram_tensor("out_s", [4, blk], F32, kind="ExternalOutput")

    with tile.TileContext(nc) as tc:
        with tc.tile_pool(name="persist", bufs=1) as persist:
            xt_t = [persist.tile([P, xcols], BF16, name=f"xt{c}")
                    for c in range(emb_c)]
            wq_sb = persist.tile([P, emb_c, DK], BF16)
            wk_sb = persist.tile([P, emb_c, DK], BF16)
            wv_sb = persist.tile([P, emb_c, DK], BF16)
            qt_sb = persist.tile([P, 4 * blk], BF16)
            kt_sb = persist.tile([P, 8 * blk], BF16)
            v_sb = persist.tile([P, 8 * sub, P], BF16)
            vt_all = persist.tile([P, xcols], BF16)
            dmask = persist.tile([P, sub, blk], BF16)
            ones_sb = persist.tile([P, 1], BF16)
            pad_sb = persist.tile([P, 1], F32)
            sums_sb = persist.tile([1, 4 * blk], F32)

            # ---- input DMAs.  dma_start blocks the ISSUING engine on ring
            # backpressure, so the Scalar engine gets ZERO input DMAs (it
            # must reach the exp stream early).  Sync (HWDGE) carries the
            # startup-critical own-half stream + peer groups 4/5; GpSimd
            # (SWDGE, otherwise idle) carries peer groups 6/7 + pad.
            # chunk-0 weights + first xt pieces gate the first matmul on
            # ~160KB, not ~800KB.
            nc.sync.dma_start(wk_sb[:, 0, :], wk.ap()[:, 0, :])
            nc.scalar.dma_start(wv_sb[:, 0, :], wv.ap()[:, 0, :])
            nc.sync.dma_start(xt_t[0][:, 0:blk], xt.ap()[0:P, 0:blk])
            nc.scalar.dma_start(xt_t[0][:, blk:2 * blk],
                                xt.ap()[0:P, blk:2 * blk])
            nc.sync.dma_start(xt_t[0][:, 2 * blk:kcols],
                              xt.ap()[0:P, 2 * blk:kcols])
            nc.sync.dma_start(wk_sb[:, 1, :], wk.ap()[:, 1, :])
            nc.scalar.dma_start(wv_sb[:, 1, :], wv.ap()[:, 1, :])
            nc.scalar.dma_start(xt_t[1][:, 0:2 * blk],
                                xt.ap()[P:2 * P, 0:2 * blk])
            nc.sync.dma_start(xt_t[1][:, 2 * blk:kcols],
                              xt.ap()[P:2 * P, 2 * blk:kcols])
            nc.sync.dma_start(wk_sb[:, 2:emb_c, :], wk.ap()[:, 2:emb_c, :])
            nc.scalar.dma_start(wv_sb[:, 2:emb_c, :], wv.ap()[:, 2:emb_c, :])
            # each chunk split across BOTH rings so it arrives at the
            # combined bandwidth (the scalar/ACT ring has fewer HW engines)
            own_last = None
            for c in range(2, emb_c):
                own_last = nc.sync.dma_start(
                    xt_t[c][:, 0:2 * blk],
                    xt.ap()[c * P:(c + 1) * P, 0:2 * blk])
                nc.scalar.dma_start(xt_t[c][:, 2 * blk:kcols],
                                    xt.ap()[c * P:(c + 1) * P,
                                            2 * blk:kcols])
            nc.sync.dma_start(wq_sb[:], wq.ap())
            # peer half, one [P, 1024] piece per chunk per engine:
            # groups 4+5 (sync, needed first), groups 6+7 (gpsimd SWDGE).
            # The gpsimd DMAs are explicitly gated on the last own-half
            # chunk so they don't steal HBM bandwidth from the critical
            # own-half stream (they'd otherwise start immediately).
            for c in range(emb_c):
                nc.sync.dma_start(xt_t[c][:, kcols:kcols + 2 * blk],
                                  xt.ap()[c * P:(c + 1) * P,
                                          kcols:kcols + 2 * blk])
            nc.gpsimd.dma_start(pad_sb[:], pad.ap())
            for c in range(emb_c):
                peer_dma = nc.gpsimd.dma_start(
                    xt_t[c][:, kcols + 2 * blk:xcols],
                    xt.ap()[c * P:(c + 1) * P,
                            kcols + 2 * blk:xcols])
                if c == 0 and own_last is not None:
                    add_dep_helper(
                        peer_dma.ins, own_last.ins, sync=True,
                        reason="delay peer-67 DMA behind own-half stream")
            # constants / masks on gpsimd (cheap, before its DMAs matter)
            nc.gpsimd.memset(ones_sb[:], 1.0)
            nc.gpsimd.memset(dmask[:], 1.0)
            for j in range(sub):
                nc.gpsimd.affine_select(
                    out=dmask[:, j, :],
                    in_=dmask[:, j, :],
                    compare_op=mybir.AluOpType.is_ge,
                    fill=0.0,
                    base=-(j * P),
                    pattern=[[1, blk]],
                    channel_multiplier=-1,
                )

            # ---- own-half K^T/V^T and Q^T projections (8-bank PSUM pool,
            # per-bank tag reuse for precise WAR deps).  Chunk-outer so the
            # PE trails the xt DMA arrivals; V^T -> natural V via the DMA
            # crossbar transpose.
            with tc.tile_pool(name="proj_psum", bufs=1, space="PSUM") as pp:
                k_ps = [pp.tile([P, blk], F32, tag=f"pk{n}", name=f"kps_{n}")
                        for n in range(nch)]
                v_ps = [pp.tile([P, blk], F32, tag=f"pv{n}", name=f"vps_{n}")
                        for n in range(nch)]
                for c in range(emb_c):
                    for n in range(nch):
                        nc.tensor.matmul(
                            k_ps[n][:], wk_sb[:, c, :],
                            xt_t[c][:, n * blk:(n + 1) * blk],
                            start=(c == 0), stop=(c == emb_c - 1))
                    for n in range(nch):
                        nc.tensor.matmul(
                            v_ps[n][:], wv_sb[:, c, :],
                            xt_t[c][:, n * blk:(n + 1) * blk],
                            start=(c == 0), stop=(c == emb_c - 1))
                # PSUM -> SBUF copies spread over two engines (Scalar is
                # still free of exp work here)
                for n in range(nch):
                    dst = kt_sb[:, n * blk:(n + 1) * blk]
                    if n % 2 == 0:
                        nc.scalar.copy(dst, k_ps[n][:])
                    else:
                        nc.vector.tensor_copy(dst, k_ps[n][:])
                for n in range(nch):
                    dst = vt_all[:, n * blk:(n + 1) * blk]
                    if n % 2 == 0:
                        nc.scalar.copy(dst, v_ps[n][:])
                    else:
                        nc.vector.tensor_copy(dst, v_ps[n][:])
                    nc.sync.dma_start_transpose(
                        v_sb[:, n * sub:(n + 1) * sub, :], dst)
                # Q tile 0 only -- reuses the K0 bank (per-bank WAR dep).
                # Q tiles 1..3 are projected later as attention-phase
                # filler bundles so tile-0's first St/exp start ASAP.
                q_ps0 = pp.tile([P, blk], F32, tag="pk0", name="qps_0")
                for c in range(emb_c):
                    nc.tensor.matmul(q_ps0[:], wq_sb[:, c, :],
                                     xt_t[c][:, 0:blk],
                                     start=(c == 0),
                                     stop=(c == emb_c - 1))
                nc.scalar.copy(qt_sb[:, 0:blk], q_ps0[:])

            # ---- attention tiles 0..3 with peer-group projections
            # pipelined in as PE filler ----
            halves = 2
            hs = sub // halves
            with (
                tc.tile_pool(name="st_psum", bufs=2, space="PSUM") as stp,
                tc.tile_pool(name="ot_psum", bufs=1, space="PSUM") as otp,
                tc.tile_pool(name="sum_psum", bufs=1, space="PSUM") as smp,
                tc.tile_pool(name="pj_psum", bufs=2, space="PSUM") as pjp,
                tc.tile_pool(name="pt_pool", bufs=4) as ptp,
                tc.tile_pool(name="acc_pool", bufs=4) as accp,
                tc.tile_pool(name="ot_sb_pool", bufs=2) as osp,
            ):
                def mm_bundle(w_sb, g, out_fn, nm):
                    """8 chunk-matmuls accumulating x[group g cols] @ W into
                    a rotating pj PSUM bank, then out_fn drains it."""
                    th = []
                    pj = pjp.tile([P, blk], F32, tag="pj", name=nm)

                    def mm(c, t=pj):
                        nc.tensor.matmul(
                            t[:], w_sb[:, c, :],
                            xt_t[c][:, g * blk:(g + 1) * blk],
                            start=(c == 0), stop=(c == emb_c - 1))
                    for c in range(emb_c):
                        th.append(lambda c=c: mm(c))
                    th.append(lambda t=pj: out_fn(t))
                    return th

                def k_bundle(g):
                    def out(t):
                        nc.vector.tensor_copy(
                            kt_sb[:, g * blk:(g + 1) * blk], t[:])
                    return mm_bundle(wk_sb, g, out, f"pjk{g}")

                def v_bundle(g):
                    def out(t):
                        dst = vt_all[:, g * blk:(g + 1) * blk]
                        nc.vector.tensor_copy(dst, t[:])
                        nc.sync.dma_start_transpose(
                            v_sb[:, g * sub:(g + 1) * sub, :], dst)
                    return mm_bundle(wv_sb, g, out, f"pjv{g}")

                def q_bundle(n):
                    def out(t):
                        nc.vector.tensor_copy(
                            qt_sb[:, n * blk:(n + 1) * blk], t[:])
                    return mm_bundle(wq_sb, n, out, f"pjq{n}")

                # Pipelined projection bundles (9 thunks each), consumed as
                # PE filler between each slot's St and AV.  Deadlines:
                # g4 K/V before tile0 slot s4; Q n before tile n; g(4+i)
                # before tile i's last slot.  All comfortably met below.
                pending = []
                for bnd in (k_bundle(4), v_bundle(4), q_bundle(1),
                            k_bundle(5), v_bundle(5), q_bundle(2),
                            k_bundle(6), v_bundle(6), q_bundle(3),
                            k_bundle(7), v_bundle(7)):
                    pending.extend(bnd)
                fill_plan = {(0, 0): 18, (0, 1): 18,
                             (1, 0): 9, (1, 1): 9, (1, 2): 9, (1, 3): 9,
                             (2, 0): 9, (2, 1): 9, (2, 2): 9}

                def emit_fill(i, si):
                    n = fill_plan.get((i, si), 0)
                    for _ in range(min(n, len(pending))):
                        pending.pop(0)()

                plans = {0: [1, 1], 1: [4], 2: [4, 2], 3: [4, 2, 2]}
                for i in (0, 1, 2, 3):
                    slots = list(range(0, i + 1)) + list(range(4, 5 + i))
                    ot = otp.tile([P, blk], F32, tag="ot", name=f"ot_{i}")
                    sm = smp.tile([1, blk], F32, tag="sm", name=f"sm_{i}")
                    n_mm = 2 * (i + 1) * sub
                    mm = 0
                    qs = qt_sb[:, i * blk:(i + 1) * blk]
                    accs = []
                    sm_n = 0
                    plan = plans[i]
                    bi = 0
                    for si, s in enumerate(slots):
                        pts = []
                        diag = (s == i)
                        for h in range(halves):
                            st = stp.tile([P, hs * blk], F32, tag="st")
                            for j in range(hs):
                                jj = h * hs + j
                                # on the diagonal group only q >= key is
                                # live; skip the upper-triangle columns
                                off = jj * P if diag else 0
                                nc.tensor.matmul(
                                    st[:, j * blk + off:(j + 1) * blk],
                                    kt_sb[:, s * blk + jj * P:
                                          s * blk + (jj + 1) * P],
                                    qs[:, off:blk],
                                    start=True, stop=True)
                            pt = ptp.tile([P, hs * blk], BF16, tag="pt")
                            if diag:
                                for j in range(hs):
                                    off = (h * hs + j) * P
                                    nc.scalar.activation(
                                        pt[:, j * blk + off:(j + 1) * blk],
                                        st[:, j * blk + off:(j + 1) * blk],
                                        mybir.ActivationFunctionType.Exp,
                                        bias=0.0, scale=SCALE)
                                # the tri mask also zeroes the stale
                                # (skipped) upper-triangle region of pt
                                nc.vector.tensor_tensor(
                                    pt[:], pt[:],
                                    dmask[:, h * hs:(h + 1) * hs, :]
                                    .rearrange("p s b -> p (s b)"),
                                    mybir.AluOpType.mult)
                            else:
                                # pad group (s == 4+i): bias is -1e30 on
                                # light cores -> exp == 0, which zeroes the
                                # non-causal group without a DVE multiply
                                nc.scalar.activation(
                                    pt[:], st[:],
                                    mybir.ActivationFunctionType.Exp,
                                    bias=(pad_sb[:, 0:1] if s == 4 + i
                                          else 0.0),
                                    scale=SCALE)
                            pts.append(pt)
                        # PE filler: next peer group's projection matmuls
                        # run while the Scalar engine exps this slot
                        emit_fill(i, si)
                        # Ot accumulation AFTER both halves' St+exp: keeps
                        # the scalar engine's exp stream gapless
                        for h in range(halves):
                            pt = pts[h]
                            for j in range(hs):
                                jj = h * hs + j
                                off = jj * P if diag else 0
                                nc.tensor.matmul(
                                    ot[:, off:blk],
                                    v_sb[:, s * sub + jj, :],
                                    pt[:, j * blk + off:(j + 1) * blk],
                                    start=(mm == 0),
                                    stop=(mm == n_mm - 1))
                                mm += 1
                        # row sums: DVE partial adds reduce each group to a
                        # [P, blk] tile; batches share one ones-matmul
                        # (PSUM-accumulated across the tile)
                        acc = accp.tile([P, blk], BF16, tag="acc")
                        h0, h1 = pts
                        nc.vector.tensor_tensor(
                            acc[:], h0[:, 0:blk], h0[:, blk:2 * blk],
                            mybir.AluOpType.add)
                        tmp = accp.tile([P, blk], BF16, tag="acc2")
                        nc.vector.tensor_tensor(
                            tmp[:], h1[:, 0:blk], h1[:, blk:2 * blk],
                            mybir.AluOpType.add)
                        nc.vector.tensor_tensor(
                            acc[:], acc[:], tmp[:], mybir.AluOpType.add)
                        accs.append(acc)
                        if len(accs) == plan[bi]:
                            for a in accs[1:]:
                                nc.vector.tensor_tensor(
                                    accs[0][:], accs[0][:], a[:],
                                    mybir.AluOpType.add)
                            nc.tensor.matmul(sm[:], ones_sb[:, 0:1],
                                             accs[0][:],
                                             start=(sm_n == 0),
                                             stop=(si == len(slots) - 1))
                            sm_n += 1
                            bi += 1
                            accs = []
                    nc.vector.tensor_copy(
                        sums_sb[0:1, i * blk:(i + 1) * blk], sm[:])
                    ot_out = osp.tile([P, blk], F32, tag="ot_sb")
                    if i == 3:
                        # last tile: split the drain copy across two engines
                        # so the kernel tail is short
                        hb = blk // 2
                        nc.scalar.copy(ot_out[:, 0:hb], ot[:, 0:hb])
                        nc.sync.dma_start(
                            out_o.ap()[:, i * blk:i * blk + hb],
                            ot_out[:, 0:hb])
                        nc.vector.tensor_copy(ot_out[:, hb:blk],
                                              ot[:, hb:blk])
                        nc.sync.dma_start(
                            out_o.ap()[:, i * blk + hb:(i + 1) * blk],
                            ot_out[:, hb:blk])
                    else:
                        nc.vector.tensor_copy(ot_out[:], ot[:])
                        nc.sync.dma_start(
                            out_o.ap()[:, i * blk:(i + 1) * blk], ot_out[:])
                    nc.sync.dma_start(out_s.ap()[i:i + 1, :],
                                      sums_sb[0:1, i * blk:(i + 1) * blk])

    nc.compile()
    return nc


_NC_CACHE = {}


def _get_nc(seq: int):
    if seq not in _NC_CACHE:
        _NC_CACHE[seq] = build_nc(seq)
    return _NC_CACHE[seq]


def make_in_maps(x, Wq, Wk, Wv, seq=None):
    """Host-side sharding: build the 8 per-core input maps."""
    x = np.asarray(x, dtype=np.float32)
    Wq = np.asarray(Wq, dtype=np.float32)
    Wk = np.asarray(Wk, dtype=np.float32)
    Wv = np.asarray(Wv, dtype=np.float32)
    seq = seq or x.shape[1]
    blk = seq // NBLK
    in_maps = []

    def warr(W):
        # [1024, 128] -> [P, emb_chunks, 128] so the device DMA is contiguous
        return np.ascontiguousarray(
            W.reshape(-1, P, DK).transpose(1, 0, 2)).astype(ml_dtypes.bfloat16)

    warrs = {"wq": warr(Wq), "wk": warr(Wk), "wv": warr(Wv)}
    for core in range(NCORES):
        b, h = core // 2, core % 2
        blocks = HEAVY_BLOCKS if h == 0 else LIGHT_BLOCKS
        rows = np.concatenate(
            [np.arange(g * blk, (g + 1) * blk) for g in blocks])
        peer_blocks = LIGHT_BLOCKS if h == 0 else HEAVY_BLOCKS
        rows_peer = np.concatenate(
            [np.arange(g * blk, (g + 1) * blk) for g in peer_blocks])
        all_rows = np.concatenate([rows, rows_peer])
        xt = np.ascontiguousarray(x[b].T[:, all_rows]).astype(
            ml_dtypes.bfloat16)
        padv = np.full((P, 1), 0.0 if h == 0 else -1e30, dtype=np.float32)
        in_maps.append({
            "xt": xt,
            "pad": padv,
            **warrs,
        })
    return in_maps


def unshard(results, seq=None, batch=BATCH):
    seq = seq or SEQ
    blk = seq // NBLK
    out = np.empty((batch, seq, DK), dtype=np.float32)
    for core in range(NCORES):
        b, h = core // 2, core % 2
        blocks = HEAVY_BLOCKS if h == 0 else LIGHT_BLOCKS
        oo = np.asarray(results[core]["out_o"])  # [128, 4*blk]
        ss = np.asarray(results[core]["out_s"])  # [4, blk]
        for i, g in enumerate(blocks):
            o_cols = oo[:, i * blk:(i + 1) * blk]        # [dv, blk]
            out[b, g * blk:(g + 1) * blk, :] = (o_cols / ss[i][None, :]).T
    return out


LAST_EXEC_NS = None
LAST_RESULTS = None


def kernel(x, Wq, Wk, Wv):
    global LAST_EXEC_NS, LAST_RESULTS
    x = np.asarray(x, dtype=np.float32)
    seq = x.shape[1]
    nc = _get_nc(seq)
    in_maps = make_in_maps(x, Wq, Wk, Wv, seq)
    trace = bool(os.environ.get("BASS_KERNEL_TRACE"))
    res = run_bass_kernel_spmd(nc, in_maps, core_ids=list(range(NCORES)),
                               trace=trace)
    LAST_EXEC_NS = res.exec_time_ns
    LAST_RESULTS = res
    return unshard(res.results, seq, x.shape[0])


if __name__ == "__main__":
    rng = np.random.default_rng(0)
    x = rng.standard_normal((BATCH, SEQ, EMB), dtype=np.float32)
    Wq = rng.standard_normal((EMB, DK), dtype=np.float32) / 32
    Wk = rng.standard_normal((EMB, DK), dtype=np.float32) / 32
    Wv = rng.standard_normal((EMB, DK), dtype=np.float32) / 32
    out = kernel(x, Wq, Wk, Wv)
    print("out", out.shape, out.dtype, "exec_ns", LAST_EXEC_NS)
